# revision 38
# baseline (speedup 1.0000x reference)
"""Multi-head attention with RoPE (LLaMA-style) on 8 Trainium2 NeuronCores.

Head-parallel tensor parallelism: each core computes 2 of 16 heads
(projections + flash-style attention) and a partial output projection;
the host sums the 8 per-core partials.

Fused single-pass structure per core: for each 512-row chunk sc we
stream x^T (pre-transposed on host), project q/k/v, apply RoPE with
full-tile vector ops, then run attention for the *previous* chunk so
projection matmuls fill the PE while the scalar engine drains exp's.
The two heads' score matmuls use disjoint PE row groups (contraction
64 at base partitions 0/64) so they run concurrently, and each j-chunk's
scores for both heads land in one [128, 2, 512] PSUM group consumed by
a single batched exp ACTIVATE.

Self-contained: hardcodes B=1, S=4096, D=1024, H=16, HD=64, 8 cores.
"""

import sys
import types

import ml_dtypes
import numpy as np

B, S, D, H, HD = 1, 4096, 1024, 16, 64
HALF = HD // 2
NC = 8                    # cores
HPC = H // NC             # heads per core (2)
CPC = HPC * HD            # qkv dims per core (128)
QCH = 512                 # query chunk (free dim of scores matmuls)
KCH = 128                 # key chunk (partition dim of scores matmuls)
NQC = S // QCH            # 8 query chunks
NKC = S // KCH            # 32 key chunks
P = 128
KC = D // P               # 8 contraction chunks for projections
VW = HD + 1               # v columns per head (64 dims + ones row)


def _install_ntff_shim():
    """antenv.axon_hooks isn't injected in this image; recreate it so
    run_bass_kernel_spmd(trace=True) can capture NTFF profiles."""
    if "antenv.axon_hooks" in sys.modules:
        return
    try:
        from trn_agent_boot.trn_boot import _ntff_profile_via_ctypes

        hook = _ntff_profile_via_ctypes("/opt/axon/libaxon_pjrt.so")
    except Exception:
        hook = None
    mod = types.ModuleType("antenv.axon_hooks")
    mod.get_axon_ntff_profile_hook = lambda: hook
    sys.modules["antenv.axon_hooks"] = mod


_install_ntff_shim()

import concourse.bacc as bacc  # noqa: E402
import concourse.mybir as mybir  # noqa: E402
import concourse.tile as tile  # noqa: E402
from concourse.bass_utils import run_bass_kernel_spmd  # noqa: E402


def _install_act_table_preference():
    """The act-table-load pass picks the first set containing each function,
    which alternates exp_and_others <-> natural_log and reloads tables every
    chunk.  Hiding Ln from the standalone natural_log set forces the picker
    onto natural_log_exp_and_others (contains BOTH Exp and Ln), so after one
    load every Exp/Ln activation hits the resident set.  Set ids still index
    the unmodified act_info.json list, so runtime behavior is unchanged."""
    if getattr(bacc, "_ant_act_tables_patched", False):
        return
    orig = bacc.get_activation_tables
    cache: dict = {}

    def patched(arch):
        if arch not in cache:
            t = dict(orig(arch))
            if "natural_log" in t and "natural_log_exp_and_others" in t:
                t["natural_log"] = t["natural_log"] - {
                    mybir.ActivationFunctionType.Ln
                }
            cache[arch] = t
        return cache[arch]

    bacc.get_activation_tables = patched
    bacc._ant_act_tables_patched = True


_install_act_table_preference()

F32 = mybir.dt.float32
BF16 = mybir.dt.bfloat16
AX = mybir.AluOpType

_BUILD_CACHE: dict = {}


def _build(mask_mode: str, debug: bool = False):
    """Build the per-core Bass program.  mask_mode: causal | none | general."""
    key = (mask_mode, debug)
    if key in _BUILD_CACHE:
        return _BUILD_CACHE[key]

    nc = bacc.Bacc("TRN2", target_bir_lowering=False, debug=False, num_devices=NC)

    # x^T pre-chunked on host: [p, sc, kc, t] = x[sc*512+t, kc*128+p]
    xtr = nc.dram_tensor("xtr", [P, NQC, KC, QCH], BF16, kind="ExternalInput")
    # host-packed [p, kc, c] so the load is contiguous per partition
    wqkvT = nc.dram_tensor("wqkvT", [P, KC, 3 * CPC], BF16, kind="ExternalInput")
    # per-core slice of wo_w.T (rows = this core's head dims)
    woT = nc.dram_tensor("woT", [CPC, D], BF16, kind="ExternalInput")
    # trig rows replicated per 32-row group: trigC = [cosT]*4,
    # trigSN = [-sinT, sinT, -sinT, sinT] (sign folded for the rope combine)
    trigC = nc.dram_tensor("trigC", [P, S], F32, kind="ExternalInput")
    trigSN = nc.dram_tensor("trigSN", [P, S], F32, kind="ExternalInput")
    qkb = nc.dram_tensor("qkb", [P, 2], F32, kind="ExternalInput")
    # qkb with 32-row halves swapped inside each 64-row head block
    qkbs = nc.dram_tensor("qkbs", [P, 2], F32, kind="ExternalInput")
    vbb = nc.dram_tensor("vbb", [P, CPC], F32, kind="ExternalInput")
    tri2 = None
    maskT = None
    if mask_mode == "causal":
        # [128, 2, 128]: the same 128x128 additive causal block for each head
        tri2 = nc.dram_tensor("tri2", [KCH, 2, KCH], F32, kind="ExternalInput")
    elif mask_mode == "general":
        maskT = nc.dram_tensor("maskT", [S, S], F32, kind="ExternalInput")
    # partial output (full rows; host sums the 8 per-core partials)
    y_out = nc.dram_tensor("y", [S, D], F32, kind="ExternalOutput")
    dbg = {}
    if debug:
        dbg["qt"] = nc.dram_tensor("dbg_qt", [P, QCH], BF16, kind="ExternalOutput")
        dbg["kt"] = nc.dram_tensor("dbg_kt", [P, QCH], BF16, kind="ExternalOutput")
        dbg["v"] = nc.dram_tensor("dbg_v", [P, HPC * VW], BF16, kind="ExternalOutput")
        dbg["pt"] = nc.dram_tensor(
            "dbg_pt", [P, HPC, QCH], BF16, kind="ExternalOutput"
        )
        dbg["ppv"] = nc.dram_tensor("dbg_ppv", [P, QCH], F32, kind="ExternalOutput")
        dbg["rec"] = nc.dram_tensor("dbg_rec", [1, QCH], F32, kind="ExternalOutput")
        dbg["at"] = nc.dram_tensor("dbg_at", [P, QCH], BF16, kind="ExternalOutput")

    causal = mask_mode == "causal"

    def n_j_of(qc):
        return 4 * (qc + 1) if causal else NKC

    with tile.TileContext(nc) as tc:
        with tc.tile_pool(name="consts", bufs=1) as cpool:
            kT_sb = cpool.tile([P, S], BF16)           # keys^T, rope'd
            v_sb = cpool.tile([P, NKC, HPC * VW], BF16)  # [s%128, s//128, h*(hd|1)]
            w_sb = cpool.tile([P, KC, 3 * CPC], BF16)
            woT_sb = cpool.tile([CPC, D], BF16)
            qkb_sb = cpool.tile([P, 2], F32)
            qkbs_sb = cpool.tile([P, 2], F32)
            vbb_sb = cpool.tile([P, CPC], F32)
            nc.sync.dma_start(out=w_sb[:], in_=wqkvT.ap())
            nc.sync.dma_start(out=qkb_sb[:], in_=qkb.ap())
            nc.sync.dma_start(out=qkbs_sb[:], in_=qkbs.ap())
            nc.sync.dma_start(out=vbb_sb[:], in_=vbb.ap())
            tri_sb = None
            if causal:
                tri_sb = cpool.tile([KCH, 2, KCH], F32)
                nc.sync.dma_start(out=tri_sb[:], in_=tri2.ap())
            # woT is not needed until the first wo (~40us in): load last
            nc.sync.dma_start(out=woT_sb[:], in_=woT.ap())
            for h in range(HPC):
                col = h * VW + HD
                nc.gpsimd.memset(v_sb[:, :, col : col + 1], 1.0)

            with (
                tc.tile_pool(name="xr", bufs=3) as xr_pool,
                tc.tile_pool(name="tc_", bufs=2) as tc_pool,
                tc.tile_pool(name="ts_", bufs=2) as ts_pool,
                tc.tile_pool(name="ring", bufs=3, space="PSUM") as ring_pool,
                tc.tile_pool(name="ppv", bufs=2, space="PSUM") as ppv_pool,
                tc.tile_pool(name="tt", bufs=2) as t_pool,
                tc.tile_pool(name="uu", bufs=2) as u_pool,
                tc.tile_pool(name="qT", bufs=2) as qT_pool,
                tc.tile_pool(name="pt", bufs=4) as pt_pool,
                tc.tile_pool(name="rec", bufs=2) as rec_pool,
                tc.tile_pool(name="lnz", bufs=2) as lnz_pool,
                tc.tile_pool(name="rec2", bufs=2) as rec2_pool,
                tc.tile_pool(name="bc", bufs=2) as bc_pool,
                tc.tile_pool(name="at", bufs=2) as at_pool,
                tc.tile_pool(name="ysb", bufs=4) as ysb_pool,
                tc.tile_pool(name="mload", bufs=4) as mload_pool,
            ):
                qTs = {}
                ats = {}

                def proj_units(sc):
                    """Emission units for chunk sc's projections; drained
                    between attention j-iterations to keep PE feeding ACT."""
                    ssl = slice(sc * QCH, (sc + 1) * QCH)
                    state = {}

                    def u_dma():
                        xr = xr_pool.tile([P, KC, QCH], BF16, name="xr")
                        nc.sync.dma_start(out=xr[:], in_=xtr.ap()[:, sc, :, :])
                        tgc = tc_pool.tile([P, QCH], F32, name="tgc")
                        nc.sync.dma_start(out=tgc[:], in_=trigC.ap()[:, ssl])
                        tgsn = ts_pool.tile([P, QCH], F32, name="tgsn")
                        nc.sync.dma_start(out=tgsn[:], in_=trigSN.ap()[:, ssl])
                        qTc = qT_pool.tile([P, QCH], BF16, name="qTc")
                        qTs[sc] = qTc
                        state.update(xr=xr, tgc=tgc, tgsn=tgsn, qTc=qTc)

                    def u_qk_mm(idx, klo, khi):
                        def run():
                            if idx == 0 and klo == 0:
                                state["ps2"] = ring_pool.tile(
                                    [P, 2, QCH], F32, name="ring"
                                )
                            ps, xr = state["ps2"][:, idx, :], state["xr"]
                            for kc in range(klo, khi):
                                nc.tensor.matmul(
                                    ps[:],
                                    lhsT=w_sb[:, kc, idx * CPC : (idx + 1) * CPC],
                                    rhs=xr[:, kc, :],
                                    start=(kc == 0),
                                    stop=(kc == KC - 1),
                                )
                        return run

                    def u_rope(idx):
                        def run():
                            ps = state["ps2"][:, idx, :]
                            tgc, tgsn = state["tgc"], state["tgsn"]
                            dst = state["qTc"] if idx == 0 else kT_sb
                            bcol = qkb_sb[:, idx : idx + 1]
                            bswc = qkbs_sb[:, idx : idx + 1]
                            # t = (ps+b)*cos; usw = cross-half sin product,
                            # half-swapped so the combine is one SB+SB add
                            # (PSUM operand base may differ; sign in tgsn)
                            t = t_pool.tile([P, QCH], BF16, name="t")
                            nc.vector.scalar_tensor_tensor(
                                t[:], ps[:], bcol, tgc[:], op0=AX.add, op1=AX.mult
                            )
                            usw = u_pool.tile([P, QCH], BF16, name="usw")
                            for h in range(HPC):
                                r0 = slice(64 * h, 64 * h + 32)
                                r1 = slice(64 * h + 32, 64 * h + 64)
                                nc.vector.scalar_tensor_tensor(
                                    usw[r0, :], ps[r1, :], bswc[r0, :],
                                    tgsn[r0, :], op0=AX.add, op1=AX.mult,
                                )
                                nc.vector.scalar_tensor_tensor(
                                    usw[r1, :], ps[r0, :], bswc[r1, :],
                                    tgsn[r1, :], op0=AX.add, op1=AX.mult,
                                )
                            osl = ssl if dst is kT_sb else slice(0, QCH)
                            nc.vector.tensor_add(dst[:, osl], t[:], usw[:])
                            if debug and sc == 0 and idx == 1:
                                nc.sync.dma_start(
                                    out=dbg["qt"].ap(), in_=state["qTc"][:]
                                )
                                nc.sync.dma_start(
                                    out=dbg["kt"].ap(), in_=kT_sb[:, 0:QCH]
                                )
                        return run

                    def u_v(sb):
                        def run():
                            xr = state["xr"]
                            jb = sc * 4 + sb
                            if sb % 2 == 0:
                                state["psv2"] = ring_pool.tile(
                                    [P, 2, QCH], F32, name="ring"
                                )
                            psv = state["psv2"][:, sb % 2, 0:CPC]
                            for kc in range(KC):
                                nc.tensor.matmul(
                                    psv[:],
                                    lhsT=xr[:, kc, sb * P : (sb + 1) * P],
                                    rhs=w_sb[:, kc, 2 * CPC : 3 * CPC],
                                    start=(kc == 0),
                                    stop=(kc == KC - 1),
                                )
                            nc.vector.tensor_add(
                                v_sb[:, jb, :]
                                .rearrange("p (h c) -> p h c", h=HPC)[:, :, 0:HD],
                                psv.rearrange("p (h c) -> p h c", h=HPC),
                                vbb_sb.rearrange("p (h c) -> p h c", h=HPC),
                            )
                            if debug and sc == 0 and sb == 0:
                                nc.sync.dma_start(
                                    out=dbg["v"].ap(), in_=v_sb[:, 0, :]
                                )
                        return run

                    units = [u_dma]
                    for idx in range(2):
                        units.append(u_qk_mm(idx, 0, 4))
                        units.append(u_qk_mm(idx, 4, KC))
                        units.append(u_rope(idx))
                    for sb in range(QCH // P):
                        units.append(u_v(sb))
                    return units

                def attn(qc, units=()):
                    from collections import deque

                    units = deque(units)
                    qsl = slice(qc * QCH, (qc + 1) * QCH)
                    qTc = qTs.pop(qc)
                    n_j = n_j_of(qc)
                    ppvs = [
                        ppv_pool.tile([VW, QCH], F32, name="ppv", tag="ppv")
                        for _ in range(HPC)
                    ]
                    per_j = max(1, -(-len(units) // max(1, n_j - 1)))

                    def emit_scores(j):
                        # scores for iteration j, emitted one iteration early
                        # so the PE FIFO never serializes them behind the
                        # exp-blocked PV of the previous iteration
                        ps = ring_pool.tile([P, HPC, QCH], F32, name="ring")
                        for h in range(HPC):
                            hr = slice(64 * h, 64 * h + 64)
                            nc.tensor.matmul(
                                ps[:, h, :],
                                lhsT=kT_sb[hr, j * KCH : (j + 1) * KCH],
                                rhs=qTc[hr, :],
                                start=True,
                                stop=True,
                            )
                        lo = max(0, KCH * j - QCH * qc) if causal else 0
                        if causal and KCH * j >= QCH * qc:
                            nc.vector.tensor_add(
                                ps[:, :, lo : lo + KCH],
                                ps[:, :, lo : lo + KCH],
                                tri_sb[:],
                            )
                        if mask_mode == "general":
                            mt = mload_pool.tile([KCH, QCH], F32, name="mt")
                            nc.sync.dma_start(
                                out=mt[:],
                                in_=maskT.ap()[j * KCH : (j + 1) * KCH, qsl],
                            )
                            for h in range(HPC):
                                nc.vector.tensor_add(
                                    ps[:, h, :], ps[:, h, :], mt[:]
                                )
                        return ps, lo

                    cur = emit_scores(0)
                    for j in range(n_j):
                        ps, lo = cur
                        nxt = emit_scores(j + 1) if j + 1 < n_j else None
                        for _ in range(per_j):
                            if units:
                                units.popleft()()
                        pt = pt_pool.tile([P, HPC, QCH], BF16, name="pt")
                        nc.scalar.activation(
                            pt[:, :, lo:QCH], ps[:, :, lo:QCH],
                            mybir.ActivationFunctionType.Exp, scale=0.125,
                        )
                        if debug and qc == 0 and j == 0:
                            nc.sync.dma_start(out=dbg["pt"].ap(), in_=pt[:])
                        for h in range(HPC):
                            nc.tensor.matmul(
                                ppvs[h][:, lo:QCH],
                                lhsT=v_sb[:, j, h * VW : (h + 1) * VW],
                                rhs=pt[:, h, lo:QCH],
                                start=(j == 0),
                                stop=(j == n_j - 1),
                                skip_group_check=True,
                            )
                        cur = nxt
                    while units:
                        units.popleft()()
                    # normalize:  at[h] = ppv[h][0:HD] / ppv[h][HD]
                    # 1/Z via exp(-ln Z) on ScalarE: stock reciprocal is
                    # 8 cyc/elem on DVE and the custom approx op is broken
                    # on this runtime.  Both heads' denominators collect on
                    # partition 0 (clean -64 partition shift from PSUM).
                    den2 = rec_pool.tile([1, HPC, QCH], F32, name="den2")
                    for h in range(HPC):
                        nc.vector.tensor_copy(
                            den2[0:1, h, :], ppvs[h][HD : HD + 1, :]
                        )
                    lnz = lnz_pool.tile([1, HPC, QCH], F32, name="lnz")
                    nc.scalar.activation(
                        lnz[:], den2[:], mybir.ActivationFunctionType.Ln
                    )
                    rec2 = rec2_pool.tile([1, HPC, QCH], F32, name="rec2")
                    nc.scalar.activation(
                        rec2[:], lnz[:], mybir.ActivationFunctionType.Exp,
                        scale=-1.0,
                    )
                    at = at_pool.tile([P, QCH], BF16, name="at")
                    for h in range(HPC):
                        hr = slice(64 * h, 64 * h + 64)
                        bc = bc_pool.tile([P, QCH], F32, name="bc")
                        nc.gpsimd.partition_broadcast(bc[:], rec2[0:1, h, :])
                        nc.vector.tensor_mul(at[hr, :], ppvs[h][0:HD, :], bc[hr, :])
                        if debug and qc == 0 and h == 0:
                            ptmp = ysb_pool.tile([P, QCH], F32, name="ysb")
                            nc.vector.tensor_copy(ptmp[0:VW, :], ppvs[h][:])
                            nc.sync.dma_start(out=dbg["ppv"].ap(), in_=ptmp[:])
                            nc.sync.dma_start(out=dbg["rec"].ap(), in_=rec2[0:1, 0, :])
                    if debug and qc == 0:
                        nc.sync.dma_start(out=dbg["at"].ap(), in_=at[:])
                    ats[qc] = at

                def wo_units(qc):
                    # partial output projection, emitted a chunk late and
                    # interleaved so the PE never stalls on the normalize chain
                    at = ats.pop(qc)
                    units = []
                    for sb in range(QCH // P):
                        row0 = qc * QCH + sb * P

                        def run(sb=sb, row0=row0):
                            psy2 = ring_pool.tile([P, 2, 512], F32, name="ring")
                            for nn in range(D // 512):
                                nc.tensor.matmul(
                                    psy2[:, nn, :],
                                    lhsT=at[:, sb * P : (sb + 1) * P],
                                    rhs=woT_sb[:, nn * 512 : (nn + 1) * 512],
                                    start=True,
                                    stop=True,
                                )
                            ysb = ysb_pool.tile([P, D], F32, name="ysb")
                            nc.vector.tensor_copy(ysb[:], psy2[:])
                            nc.sync.dma_start(
                                out=y_out.ap()[row0 : row0 + P, :], in_=ysb[:]
                            )

                        units.append(run)
                    return units

                for u in proj_units(0):
                    u()
                for qc in range(NQC):
                    units = []
                    if qc + 1 < NQC:
                        units += proj_units(qc + 1)
                    if qc >= 1:
                        units += wo_units(qc - 1)
                    attn(qc, units)
                for u in wo_units(NQC - 1):
                    u()

    nc.compile()
    _BUILD_CACHE[key] = nc
    return nc


def _detect_mask_mode(mask: np.ndarray):
    m = np.asarray(mask, np.float32).reshape(S, S)
    if not m.any():
        return "none", 0.0, m
    mval = float(m[0, 1])
    if mval < -1e8 and np.array_equal(
        m, np.triu(np.full((S, S), mval, np.float32), 1)
    ):
        return "causal", mval, m
    return "general", 0.0, m


def kernel(
    x, start_pos, freqs_cos, freqs_sin, mask,
    wq_w, wq_b, wk_w, wk_b, wv_w, wv_b, wo_w, wo_b,
):
    x = np.asarray(x, np.float32).reshape(S, D)
    freqs_cos = np.asarray(freqs_cos, np.float32)
    freqs_sin = np.asarray(freqs_sin, np.float32)
    mask_mode, mval, m2d = _detect_mask_mode(np.asarray(mask))

    # pair-split permutation within each head: [0,2,..,62, 1,3,..,63]
    perm1 = np.concatenate([np.arange(0, HD, 2), np.arange(1, HD, 2)])
    perm = np.concatenate([perm1 + h * HD for h in range(HPC)])

    # x^T pre-chunked: [p, sc, kc, t] = x[sc*512+t, kc*128+p]
    xtr = np.ascontiguousarray(
        x.reshape(NQC, QCH, KC, P).transpose(3, 0, 2, 1)
    ).astype(ml_dtypes.bfloat16)

    # trig rows: each 32-row group is cos^T (trigC); trigSN carries the
    # combine sign: [-sin, +sin] per 64-row head block
    cosT = np.ascontiguousarray(freqs_cos.T)  # [32, S]
    sinT = np.ascontiguousarray(freqs_sin.T)
    trigC = np.concatenate([cosT] * 4, axis=0).astype(np.float32)
    trigSN = np.concatenate([-sinT, sinT] * 2, axis=0).astype(np.float32)

    woT_full = np.ascontiguousarray(np.asarray(wo_w, np.float32).T)

    tri2_np = None
    if mask_mode == "causal":
        kk = np.arange(KCH)[:, None]
        qq = np.arange(KCH)[None, :]
        tri1 = np.where(kk > qq, np.float32(8.0 * mval), np.float32(0.0)).astype(
            np.float32
        )
        tri2_np = np.ascontiguousarray(
            np.broadcast_to(tri1[:, None, :], (KCH, 2, KCH))
        )
    maskT_np = None
    if mask_mode == "general":
        maskT_np = np.ascontiguousarray((8.0 * m2d).T.astype(np.float32))

    in_maps = []
    for c in range(NC):
        rows = slice(c * CPC, (c + 1) * CPC)
        wq_s = np.asarray(wq_w, np.float32)[rows, :][perm, :]
        wk_s = np.asarray(wk_w, np.float32)[rows, :][perm, :]
        wv_s = np.asarray(wv_w, np.float32)[rows, :]
        wqkvT = np.concatenate([wq_s.T, wk_s.T, wv_s.T], axis=1).astype(
            ml_dtypes.bfloat16
        )
        # pack [D, 384] -> [p, kc, c] (contiguous per partition)
        wqkvT = wqkvT.reshape(KC, P, 3 * CPC).transpose(1, 0, 2)
        qb = np.asarray(wq_b, np.float32)[rows][perm]
        kb = np.asarray(wk_b, np.float32)[rows][perm]
        vb = np.asarray(wv_b, np.float32)[rows]
        qkb_np = np.stack([qb, kb], axis=1).astype(np.float32)
        # swap the 32-row halves within each 64-row head block
        qkbs_np = np.ascontiguousarray(
            qkb_np.reshape(HPC, 2, 32, 2)[:, ::-1].reshape(P, 2)
        )
        im = {
            "xtr": xtr,
            "wqkvT": np.ascontiguousarray(wqkvT),
            "woT": np.ascontiguousarray(woT_full[rows, :]).astype(ml_dtypes.bfloat16),
            "trigC": trigC,
            "trigSN": trigSN,
            "qkb": qkb_np,
            "qkbs": qkbs_np,
            "vbb": np.broadcast_to(vb, (P, CPC)).copy(),
        }
        if mask_mode == "causal":
            im["tri2"] = tri2_np
        elif mask_mode == "general":
            im["maskT"] = maskT_np
        in_maps.append(im)

    nc = _build(mask_mode)
    res = run_bass_kernel_spmd(nc, in_maps, list(range(NC)))
    y = np.zeros((S, D), np.float64)
    for c in range(NC):
        y += res.results[c]["y"].astype(np.float64)
    y += np.asarray(wo_b, np.float64)
    return y.reshape(B, S, D).astype(np.float32)


# revision 44
# speedup vs baseline: 1.1924x; 1.1924x over previous
"""Multi-head attention with RoPE (LLaMA-style) on 8 Trainium2 NeuronCores.

Head-parallel tensor parallelism: each core computes 2 of 16 heads
(projections + flash-style attention) and a partial output projection;
the host sums the 8 per-core partials.

Fused single-pass structure per core: for each 512-row chunk sc we
stream x^T (pre-transposed on host), project q/k/v, apply RoPE with
full-tile vector ops, then run attention for the *previous* chunk so
projection matmuls fill the PE while the scalar engine drains exp's.
The two heads' score matmuls use disjoint PE row groups (contraction
64 at base partitions 0/64) so they run concurrently, and each j-chunk's
scores for both heads land in one [128, 2, 512] PSUM group consumed by
a single batched exp ACTIVATE.

Self-contained: hardcodes B=1, S=4096, D=1024, H=16, HD=64, 8 cores.
"""

import sys
import types

import ml_dtypes
import numpy as np

B, S, D, H, HD = 1, 4096, 1024, 16, 64
HALF = HD // 2
NC = 8                    # cores
HPC = H // NC             # heads per core (2)
CPC = HPC * HD            # qkv dims per core (128)
QCH = 512                 # query chunk (free dim of scores matmuls)
KCH = 128                 # key chunk (partition dim of scores matmuls)
NQC = S // QCH            # 8 query chunks
NKC = S // KCH            # 32 key chunks
P = 128
KC = D // P               # 8 contraction chunks for projections
VW = HD + 1               # v columns per head (64 dims + ones row)


def _install_ntff_shim():
    """antenv.axon_hooks isn't injected in this image; recreate it so
    run_bass_kernel_spmd(trace=True) can capture NTFF profiles."""
    if "antenv.axon_hooks" in sys.modules:
        return
    try:
        from trn_agent_boot.trn_boot import _ntff_profile_via_ctypes

        hook = _ntff_profile_via_ctypes("/opt/axon/libaxon_pjrt.so")
    except Exception:
        hook = None
    mod = types.ModuleType("antenv.axon_hooks")
    mod.get_axon_ntff_profile_hook = lambda: hook
    sys.modules["antenv.axon_hooks"] = mod


_install_ntff_shim()

import concourse.bacc as bacc  # noqa: E402
import concourse.mybir as mybir  # noqa: E402
import concourse.tile as tile  # noqa: E402
from concourse.bass_utils import run_bass_kernel_spmd  # noqa: E402


def _install_act_table_preference():
    """The act-table-load pass picks the first set containing each function,
    which alternates exp_and_others <-> natural_log and reloads tables every
    chunk.  Hiding Ln from the standalone natural_log set forces the picker
    onto natural_log_exp_and_others (contains BOTH Exp and Ln), so after one
    load every Exp/Ln activation hits the resident set.  Set ids still index
    the unmodified act_info.json list, so runtime behavior is unchanged."""
    if getattr(bacc, "_ant_act_tables_patched", False):
        return
    orig = bacc.get_activation_tables
    cache: dict = {}

    def patched(arch):
        if arch not in cache:
            t = dict(orig(arch))
            if "natural_log" in t and "natural_log_exp_and_others" in t:
                t["natural_log"] = t["natural_log"] - {
                    mybir.ActivationFunctionType.Ln
                }
            cache[arch] = t
        return cache[arch]

    bacc.get_activation_tables = patched
    bacc._ant_act_tables_patched = True


_install_act_table_preference()

F32 = mybir.dt.float32
BF16 = mybir.dt.bfloat16
AX = mybir.AluOpType

_BUILD_CACHE: dict = {}


def _build(mask_mode: str, debug: bool = False):
    """Build the per-core Bass program.  mask_mode: causal | none | general."""
    key = (mask_mode, debug)
    if key in _BUILD_CACHE:
        return _BUILD_CACHE[key]

    nc = bacc.Bacc("TRN2", target_bir_lowering=False, debug=False, num_devices=NC)

    # x^T pre-chunked on host: [p, sc, kc, t] = x[sc*512+t, kc*128+p]
    xtr = nc.dram_tensor("xtr", [P, NQC, KC, QCH], BF16, kind="ExternalInput")
    # host-packed [p, kc, c] so the load is contiguous per partition
    wqkvT = nc.dram_tensor("wqkvT", [P, KC, 3 * CPC], BF16, kind="ExternalInput")
    # per-core slice of wo_w.T (rows = this core's head dims)
    woT = nc.dram_tensor("woT", [CPC, D], BF16, kind="ExternalInput")
    # trig rows replicated per 32-row group: trigC = [cosT]*4,
    # trigSN = [-sinT, sinT, -sinT, sinT] (sign folded for the rope combine)
    trigC = nc.dram_tensor("trigC", [P, S], F32, kind="ExternalInput")
    trigSN = nc.dram_tensor("trigSN", [P, S], F32, kind="ExternalInput")
    qkb = nc.dram_tensor("qkb", [P, 2], F32, kind="ExternalInput")
    # qkb with 32-row halves swapped inside each 64-row head block
    qkbs = nc.dram_tensor("qkbs", [P, 2], F32, kind="ExternalInput")
    vbb = nc.dram_tensor("vbb", [P, CPC], F32, kind="ExternalInput")
    tri2 = None
    maskT = None
    if mask_mode == "causal":
        # [128, 2, 128]: the same 128x128 additive causal block for each head
        tri2 = nc.dram_tensor("tri2", [KCH, 2, KCH], F32, kind="ExternalInput")
    elif mask_mode == "general":
        maskT = nc.dram_tensor("maskT", [S, S], F32, kind="ExternalInput")
    # partial output (full rows; host sums the 8 per-core partials)
    y_out = nc.dram_tensor("y", [S, D], F32, kind="ExternalOutput")
    dbg = {}
    if debug:
        dbg["qt"] = nc.dram_tensor("dbg_qt", [P, QCH], BF16, kind="ExternalOutput")
        dbg["kt"] = nc.dram_tensor("dbg_kt", [P, QCH], BF16, kind="ExternalOutput")
        dbg["v"] = nc.dram_tensor("dbg_v", [P, HPC * VW], BF16, kind="ExternalOutput")
        dbg["pt"] = nc.dram_tensor(
            "dbg_pt", [P, HPC, QCH], BF16, kind="ExternalOutput"
        )
        dbg["ppv"] = nc.dram_tensor("dbg_ppv", [P, QCH], F32, kind="ExternalOutput")
        dbg["rec"] = nc.dram_tensor("dbg_rec", [1, QCH], F32, kind="ExternalOutput")
        dbg["at"] = nc.dram_tensor("dbg_at", [P, QCH], BF16, kind="ExternalOutput")

    causal = mask_mode == "causal"

    def n_j_of(qc):
        return 4 * (qc + 1) if causal else NKC

    with tile.TileContext(nc) as tc:
        with tc.tile_pool(name="consts", bufs=1) as cpool:
            kT_sb = cpool.tile([P, S], BF16)           # keys^T, rope'd
            v_sb = cpool.tile([P, NKC, HPC * VW], BF16)  # [s%128, s//128, h*(hd|1)]
            w_sb = cpool.tile([P, KC, 3 * CPC], BF16)
            woT_sb = cpool.tile([CPC, D], BF16)
            qkb_sb = cpool.tile([P, 2], F32)
            qkbs_sb = cpool.tile([P, 2], F32)
            vbb_sb = cpool.tile([P, CPC], F32)
            nc.sync.dma_start(out=w_sb[:], in_=wqkvT.ap())
            nc.sync.dma_start(out=qkb_sb[:], in_=qkb.ap())
            nc.sync.dma_start(out=qkbs_sb[:], in_=qkbs.ap())
            nc.sync.dma_start(out=vbb_sb[:], in_=vbb.ap())
            tri_sb = None
            if causal:
                tri_sb = cpool.tile([KCH, 2, KCH], F32)
                nc.sync.dma_start(out=tri_sb[:], in_=tri2.ap())
            # woT is not needed until the first wo (~40us in): load last
            nc.sync.dma_start(out=woT_sb[:], in_=woT.ap())
            for h in range(HPC):
                col = h * VW + HD
                nc.gpsimd.memset(v_sb[:, :, col : col + 1], 1.0)

            with (
                tc.tile_pool(name="xr", bufs=3) as xr_pool,
                tc.tile_pool(name="tc_", bufs=2) as tc_pool,
                tc.tile_pool(name="ts_", bufs=2) as ts_pool,
                tc.tile_pool(name="pps", bufs=2, space="PSUM") as pps_pool,
                tc.tile_pool(name="scp", bufs=2, space="PSUM") as sc_pool,
                tc.tile_pool(name="ppv", bufs=2, space="PSUM") as ppv_pool,
                tc.tile_pool(name="tt", bufs=2) as t_pool,
                tc.tile_pool(name="uu", bufs=2) as u_pool,
                tc.tile_pool(name="qT", bufs=2) as qT_pool,
                tc.tile_pool(name="pt", bufs=4) as pt_pool,
                tc.tile_pool(name="rec", bufs=2) as rec_pool,
                tc.tile_pool(name="lnz", bufs=2) as lnz_pool,
                tc.tile_pool(name="rec2", bufs=2) as rec2_pool,
                tc.tile_pool(name="bc", bufs=2) as bc_pool,
                tc.tile_pool(name="at", bufs=2) as at_pool,
                tc.tile_pool(name="ysb", bufs=4) as ysb_pool,
                tc.tile_pool(name="mload", bufs=4) as mload_pool,
            ):
                qTs = {}
                ats = {}

                def proj_units(sc):
                    """Emission units for chunk sc's projections; drained
                    between attention j-iterations to keep PE feeding ACT."""
                    ssl = slice(sc * QCH, (sc + 1) * QCH)
                    state = {}

                    def u_dma():
                        xr = xr_pool.tile([P, KC, QCH], BF16, name="xr")
                        nc.sync.dma_start(out=xr[:], in_=xtr.ap()[:, sc, :, :])
                        tgc = tc_pool.tile([P, QCH], F32, name="tgc")
                        nc.sync.dma_start(out=tgc[:], in_=trigC.ap()[:, ssl])
                        tgsn = ts_pool.tile([P, QCH], F32, name="tgsn")
                        nc.sync.dma_start(out=tgsn[:], in_=trigSN.ap()[:, ssl])
                        qTc = qT_pool.tile([P, QCH], BF16, name="qTc")
                        qTs[sc] = qTc
                        state.update(xr=xr, tgc=tgc, tgsn=tgsn, qTc=qTc)

                    def u_qk_mm(idx, klo, khi):
                        def run():
                            if klo == 0:
                                state["ps"] = pps_pool.tile(
                                    [P, QCH], F32, name="pps"
                                )
                            ps, xr = state["ps"], state["xr"]
                            for kc in range(klo, khi):
                                nc.tensor.matmul(
                                    ps[:],
                                    lhsT=w_sb[:, kc, idx * CPC : (idx + 1) * CPC],
                                    rhs=xr[:, kc, :],
                                    start=(kc == 0),
                                    stop=(kc == KC - 1),
                                )
                        return run

                    def u_rope(idx):
                        def run():
                            ps = state["ps"]
                            tgc, tgsn = state["tgc"], state["tgsn"]
                            dst = state["qTc"] if idx == 0 else kT_sb
                            bcol = qkb_sb[:, idx : idx + 1]
                            bswc = qkbs_sb[:, idx : idx + 1]
                            # t = (ps+b)*cos; usw = cross-half sin product,
                            # half-swapped so the combine is one SB+SB add
                            # (PSUM operand base may differ; sign in tgsn)
                            t = t_pool.tile([P, QCH], BF16, name="t")
                            nc.vector.scalar_tensor_tensor(
                                t[:], ps[:], bcol, tgc[:], op0=AX.add, op1=AX.mult
                            )
                            usw = u_pool.tile([P, QCH], BF16, name="usw")
                            for h in range(HPC):
                                r0 = slice(64 * h, 64 * h + 32)
                                r1 = slice(64 * h + 32, 64 * h + 64)
                                nc.vector.scalar_tensor_tensor(
                                    usw[r0, :], ps[r1, :], bswc[r0, :],
                                    tgsn[r0, :], op0=AX.add, op1=AX.mult,
                                )
                                nc.vector.scalar_tensor_tensor(
                                    usw[r1, :], ps[r0, :], bswc[r1, :],
                                    tgsn[r1, :], op0=AX.add, op1=AX.mult,
                                )
                            osl = ssl if dst is kT_sb else slice(0, QCH)
                            nc.vector.tensor_add(dst[:, osl], t[:], usw[:])
                            if debug and sc == 0 and idx == 1:
                                nc.sync.dma_start(
                                    out=dbg["qt"].ap(), in_=state["qTc"][:]
                                )
                                nc.sync.dma_start(
                                    out=dbg["kt"].ap(), in_=kT_sb[:, 0:QCH]
                                )
                        return run

                    def u_v(sb):
                        def run():
                            xr = state["xr"]
                            jb = sc * 4 + sb
                            psv = pps_pool.tile([P, CPC], F32, name="pps")
                            for kc in range(KC):
                                nc.tensor.matmul(
                                    psv[:],
                                    lhsT=xr[:, kc, sb * P : (sb + 1) * P],
                                    rhs=w_sb[:, kc, 2 * CPC : 3 * CPC],
                                    start=(kc == 0),
                                    stop=(kc == KC - 1),
                                )
                            nc.vector.tensor_add(
                                v_sb[:, jb, :]
                                .rearrange("p (h c) -> p h c", h=HPC)[:, :, 0:HD],
                                psv.rearrange("p (h c) -> p h c", h=HPC),
                                vbb_sb.rearrange("p (h c) -> p h c", h=HPC),
                            )
                            if debug and sc == 0 and sb == 0:
                                nc.sync.dma_start(
                                    out=dbg["v"].ap(), in_=v_sb[:, 0, :]
                                )
                        return run

                    units = [u_dma]
                    for idx in range(2):
                        units.append(u_qk_mm(idx, 0, 4))
                        units.append(u_qk_mm(idx, 4, KC))
                        units.append(u_rope(idx))
                    for sb in range(QCH // P):
                        units.append(u_v(sb))
                    return units

                def attn(qc, units=()):
                    from collections import deque

                    units = deque(units)
                    qsl = slice(qc * QCH, (qc + 1) * QCH)
                    qTc = qTs.pop(qc)
                    n_j = n_j_of(qc)
                    ppvs = [
                        ppv_pool.tile([VW, QCH], F32, name="ppv", tag="ppv")
                        for _ in range(HPC)
                    ]
                    per_j = max(1, -(-len(units) // max(1, n_j - 1)))

                    def emit_scores(j):
                        # scores for iteration j, emitted one iteration early
                        # so the PE FIFO never serializes them behind the
                        # exp-blocked PV of the previous iteration
                        ps = sc_pool.tile([P, HPC, QCH], F32, name="ps")
                        for h in range(HPC):
                            hr = slice(64 * h, 64 * h + 64)
                            nc.tensor.matmul(
                                ps[:, h, :],
                                lhsT=kT_sb[hr, j * KCH : (j + 1) * KCH],
                                rhs=qTc[hr, :],
                                start=True,
                                stop=True,
                            )
                        lo = max(0, KCH * j - QCH * qc) if causal else 0
                        if causal and KCH * j >= QCH * qc:
                            nc.vector.tensor_add(
                                ps[:, :, lo : lo + KCH],
                                ps[:, :, lo : lo + KCH],
                                tri_sb[:],
                            )
                        if mask_mode == "general":
                            mt = mload_pool.tile([KCH, QCH], F32, name="mt")
                            nc.sync.dma_start(
                                out=mt[:],
                                in_=maskT.ap()[j * KCH : (j + 1) * KCH, qsl],
                            )
                            for h in range(HPC):
                                nc.vector.tensor_add(
                                    ps[:, h, :], ps[:, h, :], mt[:]
                                )
                        return ps, lo

                    cur = emit_scores(0)
                    for j in range(n_j):
                        ps, lo = cur
                        nxt = emit_scores(j + 1) if j + 1 < n_j else None
                        for _ in range(per_j):
                            if units:
                                units.popleft()()
                        pt = pt_pool.tile([P, HPC, QCH], BF16, name="pt")
                        nc.scalar.activation(
                            pt[:, :, lo:QCH], ps[:, :, lo:QCH],
                            mybir.ActivationFunctionType.Exp, scale=0.125,
                        )
                        if debug and qc == 0 and j == 0:
                            nc.sync.dma_start(out=dbg["pt"].ap(), in_=pt[:])
                        for h in range(HPC):
                            nc.tensor.matmul(
                                ppvs[h][:, lo:QCH],
                                lhsT=v_sb[:, j, h * VW : (h + 1) * VW],
                                rhs=pt[:, h, lo:QCH],
                                start=(j == 0),
                                stop=(j == n_j - 1),
                                skip_group_check=True,
                            )
                        cur = nxt
                    while units:
                        units.popleft()()
                    # normalize:  at[h] = ppv[h][0:HD] / ppv[h][HD]
                    # 1/Z via exp(-ln Z) on ScalarE: stock reciprocal is
                    # 8 cyc/elem on DVE and the custom approx op is broken
                    # on this runtime.  Both heads' denominators collect on
                    # partition 0 (clean -64 partition shift from PSUM).
                    den2 = rec_pool.tile([1, HPC, QCH], F32, name="den2")
                    for h in range(HPC):
                        nc.vector.tensor_copy(
                            den2[0:1, h, :], ppvs[h][HD : HD + 1, :]
                        )
                    lnz = lnz_pool.tile([1, HPC, QCH], F32, name="lnz")
                    nc.scalar.activation(
                        lnz[:], den2[:], mybir.ActivationFunctionType.Ln
                    )
                    rec2 = rec2_pool.tile([1, HPC, QCH], F32, name="rec2")
                    nc.scalar.activation(
                        rec2[:], lnz[:], mybir.ActivationFunctionType.Exp,
                        scale=-1.0,
                    )
                    at = at_pool.tile([P, QCH], BF16, name="at")
                    for h in range(HPC):
                        hr = slice(64 * h, 64 * h + 64)
                        bc = bc_pool.tile([P, QCH], F32, name="bc")
                        nc.gpsimd.partition_broadcast(bc[:], rec2[0:1, h, :])
                        nc.vector.tensor_mul(at[hr, :], ppvs[h][0:HD, :], bc[hr, :])
                        if debug and qc == 0 and h == 0:
                            ptmp = ysb_pool.tile([P, QCH], F32, name="ysb")
                            nc.vector.tensor_copy(ptmp[0:VW, :], ppvs[h][:])
                            nc.sync.dma_start(out=dbg["ppv"].ap(), in_=ptmp[:])
                            nc.sync.dma_start(out=dbg["rec"].ap(), in_=rec2[0:1, 0, :])
                    if debug and qc == 0:
                        nc.sync.dma_start(out=dbg["at"].ap(), in_=at[:])
                    ats[qc] = at

                def wo_units(qc):
                    # partial output projection, emitted a chunk late and
                    # interleaved so the PE never stalls on the normalize chain
                    at = ats.pop(qc)
                    units = []
                    for sb in range(QCH // P):
                        row0 = qc * QCH + sb * P

                        def run(sb=sb, row0=row0):
                            ysb = ysb_pool.tile([P, D], F32, name="ysb")
                            for nn in range(D // 512):
                                psy = pps_pool.tile([P, 512], F32, name="pps")
                                nc.tensor.matmul(
                                    psy[:],
                                    lhsT=at[:, sb * P : (sb + 1) * P],
                                    rhs=woT_sb[:, nn * 512 : (nn + 1) * 512],
                                    start=True,
                                    stop=True,
                                )
                                nc.vector.tensor_copy(
                                    ysb[:, nn * 512 : (nn + 1) * 512], psy[:]
                                )
                            nc.sync.dma_start(
                                out=y_out.ap()[row0 : row0 + P, :], in_=ysb[:]
                            )

                        units.append(run)
                    return units

                for u in proj_units(0):
                    u()
                for qc in range(NQC):
                    units = []
                    if qc + 1 < NQC:
                        units += proj_units(qc + 1)
                    if qc >= 1:
                        units += wo_units(qc - 1)
                    attn(qc, units)
                for u in wo_units(NQC - 1):
                    u()

    nc.compile()
    _BUILD_CACHE[key] = nc
    return nc


def _detect_mask_mode(mask: np.ndarray):
    m = np.asarray(mask, np.float32).reshape(S, S)
    if not m.any():
        return "none", 0.0, m
    mval = float(m[0, 1])
    if mval < -1e8 and np.array_equal(
        m, np.triu(np.full((S, S), mval, np.float32), 1)
    ):
        return "causal", mval, m
    return "general", 0.0, m


def kernel(
    x, start_pos, freqs_cos, freqs_sin, mask,
    wq_w, wq_b, wk_w, wk_b, wv_w, wv_b, wo_w, wo_b,
):
    x = np.asarray(x, np.float32).reshape(S, D)
    freqs_cos = np.asarray(freqs_cos, np.float32)
    freqs_sin = np.asarray(freqs_sin, np.float32)
    mask_mode, mval, m2d = _detect_mask_mode(np.asarray(mask))

    # pair-split permutation within each head: [0,2,..,62, 1,3,..,63]
    perm1 = np.concatenate([np.arange(0, HD, 2), np.arange(1, HD, 2)])
    perm = np.concatenate([perm1 + h * HD for h in range(HPC)])

    # x^T pre-chunked: [p, sc, kc, t] = x[sc*512+t, kc*128+p]
    xtr = np.ascontiguousarray(
        x.reshape(NQC, QCH, KC, P).transpose(3, 0, 2, 1)
    ).astype(ml_dtypes.bfloat16)

    # trig rows: each 32-row group is cos^T (trigC); trigSN carries the
    # combine sign: [-sin, +sin] per 64-row head block
    cosT = np.ascontiguousarray(freqs_cos.T)  # [32, S]
    sinT = np.ascontiguousarray(freqs_sin.T)
    trigC = np.concatenate([cosT] * 4, axis=0).astype(np.float32)
    trigSN = np.concatenate([-sinT, sinT] * 2, axis=0).astype(np.float32)

    woT_full = np.ascontiguousarray(np.asarray(wo_w, np.float32).T)

    tri2_np = None
    if mask_mode == "causal":
        kk = np.arange(KCH)[:, None]
        qq = np.arange(KCH)[None, :]
        tri1 = np.where(kk > qq, np.float32(8.0 * mval), np.float32(0.0)).astype(
            np.float32
        )
        tri2_np = np.ascontiguousarray(
            np.broadcast_to(tri1[:, None, :], (KCH, 2, KCH))
        )
    maskT_np = None
    if mask_mode == "general":
        maskT_np = np.ascontiguousarray((8.0 * m2d).T.astype(np.float32))

    in_maps = []
    for c in range(NC):
        rows = slice(c * CPC, (c + 1) * CPC)
        wq_s = np.asarray(wq_w, np.float32)[rows, :][perm, :]
        wk_s = np.asarray(wk_w, np.float32)[rows, :][perm, :]
        wv_s = np.asarray(wv_w, np.float32)[rows, :]
        wqkvT = np.concatenate([wq_s.T, wk_s.T, wv_s.T], axis=1).astype(
            ml_dtypes.bfloat16
        )
        # pack [D, 384] -> [p, kc, c] (contiguous per partition)
        wqkvT = wqkvT.reshape(KC, P, 3 * CPC).transpose(1, 0, 2)
        qb = np.asarray(wq_b, np.float32)[rows][perm]
        kb = np.asarray(wk_b, np.float32)[rows][perm]
        vb = np.asarray(wv_b, np.float32)[rows]
        qkb_np = np.stack([qb, kb], axis=1).astype(np.float32)
        # swap the 32-row halves within each 64-row head block
        qkbs_np = np.ascontiguousarray(
            qkb_np.reshape(HPC, 2, 32, 2)[:, ::-1].reshape(P, 2)
        )
        im = {
            "xtr": xtr,
            "wqkvT": np.ascontiguousarray(wqkvT),
            "woT": np.ascontiguousarray(woT_full[rows, :]).astype(ml_dtypes.bfloat16),
            "trigC": trigC,
            "trigSN": trigSN,
            "qkb": qkb_np,
            "qkbs": qkbs_np,
            "vbb": np.broadcast_to(vb, (P, CPC)).copy(),
        }
        if mask_mode == "causal":
            im["tri2"] = tri2_np
        elif mask_mode == "general":
            im["maskT"] = maskT_np
        in_maps.append(im)

    nc = _build(mask_mode)
    res = run_bass_kernel_spmd(nc, in_maps, list(range(NC)))
    y = np.zeros((S, D), np.float64)
    for c in range(NC):
        y += res.results[c]["y"].astype(np.float64)
    y += np.asarray(wo_b, np.float64)
    return y.reshape(B, S, D).astype(np.float32)


# revision 49
# speedup vs baseline: 1.2921x; 1.0836x over previous
"""Multi-head attention with RoPE (LLaMA-style) on 8 Trainium2 NeuronCores.

Head-parallel tensor parallelism: each core computes 2 of 16 heads
(projections + flash-style attention) and a partial output projection;
the host sums the 8 per-core partials.

Fused single-pass structure per core: for each 512-row chunk sc we
stream x^T (pre-transposed on host), project q/k/v, apply RoPE with
full-tile vector ops, then run attention for the *previous* chunk so
projection matmuls fill the PE while the scalar engine drains exp's.
The two heads' score matmuls use disjoint PE row groups (contraction
64 at base partitions 0/64) so they run concurrently, and each j-chunk's
scores for both heads land in one [128, 2, 512] PSUM group consumed by
a single batched exp ACTIVATE.

Self-contained: hardcodes B=1, S=4096, D=1024, H=16, HD=64, 8 cores.
"""

import sys
import types

import ml_dtypes
import numpy as np

B, S, D, H, HD = 1, 4096, 1024, 16, 64
HALF = HD // 2
NC = 8                    # cores
HPC = H // NC             # heads per core (2)
CPC = HPC * HD            # qkv dims per core (128)
QCH = 512                 # query chunk (free dim of scores matmuls)
KCH = 128                 # key chunk (partition dim of scores matmuls)
NQC = S // QCH            # 8 query chunks
NKC = S // KCH            # 32 key chunks
P = 128
KC = D // P               # 8 contraction chunks for projections
VW = HD + 1               # v columns per head (64 dims + ones row)


def _install_ntff_shim():
    """antenv.axon_hooks isn't injected in this image; recreate it so
    run_bass_kernel_spmd(trace=True) can capture NTFF profiles."""
    if "antenv.axon_hooks" in sys.modules:
        return
    try:
        from trn_agent_boot.trn_boot import _ntff_profile_via_ctypes

        hook = _ntff_profile_via_ctypes("/opt/axon/libaxon_pjrt.so")
    except Exception:
        hook = None
    mod = types.ModuleType("antenv.axon_hooks")
    mod.get_axon_ntff_profile_hook = lambda: hook
    sys.modules["antenv.axon_hooks"] = mod


_install_ntff_shim()

import concourse.bacc as bacc  # noqa: E402
import concourse.mybir as mybir  # noqa: E402
import concourse.tile as tile  # noqa: E402
from concourse.bass_utils import run_bass_kernel_spmd  # noqa: E402


def _install_act_table_preference():
    """The act-table-load pass picks the first set containing each function,
    which alternates exp_and_others <-> natural_log and reloads tables every
    chunk.  Hiding Ln from the standalone natural_log set forces the picker
    onto natural_log_exp_and_others (contains BOTH Exp and Ln), so after one
    load every Exp/Ln activation hits the resident set.  Set ids still index
    the unmodified act_info.json list, so runtime behavior is unchanged."""
    if getattr(bacc, "_ant_act_tables_patched", False):
        return
    orig = bacc.get_activation_tables
    cache: dict = {}

    def patched(arch):
        if arch not in cache:
            t = dict(orig(arch))
            if "natural_log" in t and "natural_log_exp_and_others" in t:
                t["natural_log"] = t["natural_log"] - {
                    mybir.ActivationFunctionType.Ln
                }
            cache[arch] = t
        return cache[arch]

    bacc.get_activation_tables = patched
    bacc._ant_act_tables_patched = True


_install_act_table_preference()

F32 = mybir.dt.float32
BF16 = mybir.dt.bfloat16
AX = mybir.AluOpType

_BUILD_CACHE: dict = {}


def _build(mask_mode: str, debug: bool = False):
    """Build the per-core Bass program.  mask_mode: causal | none | general."""
    key = (mask_mode, debug)
    if key in _BUILD_CACHE:
        return _BUILD_CACHE[key]

    nc = bacc.Bacc("TRN2", target_bir_lowering=False, debug=False, num_devices=NC)

    # x^T pre-chunked on host: [p, sc, kc, t] = x[sc*512+t, kc*128+p]
    xtr = nc.dram_tensor("xtr", [P, NQC, KC, QCH], BF16, kind="ExternalInput")
    # host-packed [p, kc, c] so the load is contiguous per partition
    wqkvT = nc.dram_tensor("wqkvT", [P, KC, 3 * CPC], BF16, kind="ExternalInput")
    # per-core slice of wo_w.T (rows = this core's head dims)
    woT = nc.dram_tensor("woT", [CPC, D], BF16, kind="ExternalInput")
    # trig rows replicated per 32-row group: trigC = [cosT]*4,
    # trigSN = [-sinT, sinT, -sinT, sinT] (sign folded for the rope combine)
    trigC = nc.dram_tensor("trigC", [P, S], F32, kind="ExternalInput")
    trigSN = nc.dram_tensor("trigSN", [P, S], F32, kind="ExternalInput")
    qkb = nc.dram_tensor("qkb", [P, 2], F32, kind="ExternalInput")
    # qkb with 32-row halves swapped inside each 64-row head block
    qkbs = nc.dram_tensor("qkbs", [P, 2], F32, kind="ExternalInput")
    vbb = nc.dram_tensor("vbb", [P, CPC], F32, kind="ExternalInput")
    tri2 = None
    maskT = None
    if mask_mode == "causal":
        # [128, 2, 128]: the same 128x128 additive causal block for each head
        tri2 = nc.dram_tensor("tri2", [KCH, 2, KCH], F32, kind="ExternalInput")
    elif mask_mode == "general":
        maskT = nc.dram_tensor("maskT", [S, S], F32, kind="ExternalInput")
    # partial output (full rows; host sums the 8 per-core partials)
    y_out = nc.dram_tensor("y", [S, D], F32, kind="ExternalOutput")
    dbg = {}
    if debug:
        dbg["qt"] = nc.dram_tensor("dbg_qt", [P, QCH], BF16, kind="ExternalOutput")
        dbg["kt"] = nc.dram_tensor("dbg_kt", [P, QCH], BF16, kind="ExternalOutput")
        dbg["v"] = nc.dram_tensor("dbg_v", [P, HPC * VW], BF16, kind="ExternalOutput")
        dbg["pt"] = nc.dram_tensor(
            "dbg_pt", [P, HPC, QCH], BF16, kind="ExternalOutput"
        )
        dbg["ppv"] = nc.dram_tensor("dbg_ppv", [P, QCH], F32, kind="ExternalOutput")
        dbg["rec"] = nc.dram_tensor("dbg_rec", [1, QCH], F32, kind="ExternalOutput")
        dbg["at"] = nc.dram_tensor("dbg_at", [P, QCH], BF16, kind="ExternalOutput")

    causal = mask_mode == "causal"

    def n_j_of(qc):
        return 4 * (qc + 1) if causal else NKC

    with tile.TileContext(nc) as tc:
        with tc.tile_pool(name="consts", bufs=1) as cpool:
            kT_sb = cpool.tile([P, S], BF16)           # keys^T, rope'd
            v_sb = cpool.tile([P, NKC, HPC * VW], BF16)  # [s%128, s//128, h*(hd|1)]
            w_sb = cpool.tile([P, KC, 3 * CPC], BF16)
            woT_sb = cpool.tile([CPC, D], BF16)
            qkb_sb = cpool.tile([P, 2], F32)
            qkbs_sb = cpool.tile([P, 2], F32)
            vbb_sb = cpool.tile([P, CPC], F32)
            nc.sync.dma_start(out=w_sb[:], in_=wqkvT.ap())
            nc.sync.dma_start(out=qkb_sb[:], in_=qkb.ap())
            nc.sync.dma_start(out=qkbs_sb[:], in_=qkbs.ap())
            tri_sb = None
            if causal:
                tri_sb = cpool.tile([KCH, 2, KCH], F32)

            def emit_late_consts():
                # issued after the first x/trig chunk loads so the critical
                # first-matmul DMAs sit early in the queue
                if causal:
                    nc.sync.dma_start(out=tri_sb[:], in_=tri2.ap())
                nc.sync.dma_start(out=vbb_sb[:], in_=vbb.ap())
                nc.sync.dma_start(out=woT_sb[:], in_=woT.ap())

            for h in range(HPC):
                col = h * VW + HD
                nc.gpsimd.memset(v_sb[:, :, col : col + 1], 1.0)

            with (
                tc.tile_pool(name="xr", bufs=3) as xr_pool,
                tc.tile_pool(name="tc_", bufs=2) as tc_pool,
                tc.tile_pool(name="ts_", bufs=2) as ts_pool,
                tc.tile_pool(name="pps", bufs=2, space="PSUM") as pps_pool,
                tc.tile_pool(name="scp", bufs=2, space="PSUM") as sc_pool,
                tc.tile_pool(name="ppv", bufs=2, space="PSUM") as ppv_pool,
                tc.tile_pool(name="tt", bufs=2) as t_pool,
                tc.tile_pool(name="uu", bufs=2) as u_pool,
                tc.tile_pool(name="qT", bufs=2) as qT_pool,
                tc.tile_pool(name="pt", bufs=4) as pt_pool,
                tc.tile_pool(name="rec", bufs=2) as rec_pool,
                tc.tile_pool(name="lnz", bufs=2) as lnz_pool,
                tc.tile_pool(name="rec2", bufs=2) as rec2_pool,
                tc.tile_pool(name="bc", bufs=2) as bc_pool,
                tc.tile_pool(name="atu", bufs=2) as atu_pool,
                tc.tile_pool(name="at", bufs=2) as at_pool,
                tc.tile_pool(name="ysb", bufs=4) as ysb_pool,
                tc.tile_pool(name="mload", bufs=4) as mload_pool,
            ):
                qTs = {}
                ats = {}
                norms = {}

                def proj_units(sc):
                    """Emission units for chunk sc's projections; drained
                    between attention j-iterations to keep PE feeding ACT."""
                    ssl = slice(sc * QCH, (sc + 1) * QCH)
                    state = {}

                    def u_dma():
                        xr = xr_pool.tile([P, KC, QCH], BF16, name="xr")
                        nc.sync.dma_start(out=xr[:], in_=xtr.ap()[:, sc, :, :])
                        tgc = tc_pool.tile([P, QCH], F32, name="tgc")
                        nc.sync.dma_start(out=tgc[:], in_=trigC.ap()[:, ssl])
                        tgsn = ts_pool.tile([P, QCH], F32, name="tgsn")
                        nc.sync.dma_start(out=tgsn[:], in_=trigSN.ap()[:, ssl])
                        qTc = qT_pool.tile([P, QCH], BF16, name="qTc")
                        qTs[sc] = qTc
                        state.update(xr=xr, tgc=tgc, tgsn=tgsn, qTc=qTc)

                    def u_qk_mm(idx, klo, khi):
                        def run():
                            if klo == 0:
                                state["ps"] = pps_pool.tile(
                                    [P, QCH], F32, name="pps"
                                )
                            ps, xr = state["ps"], state["xr"]
                            for kc in range(klo, khi):
                                nc.tensor.matmul(
                                    ps[:],
                                    lhsT=w_sb[:, kc, idx * CPC : (idx + 1) * CPC],
                                    rhs=xr[:, kc, :],
                                    start=(kc == 0),
                                    stop=(kc == KC - 1),
                                )
                        return run

                    def u_rope(idx):
                        def run():
                            ps = state["ps"]
                            tgc, tgsn = state["tgc"], state["tgsn"]
                            dst = state["qTc"] if idx == 0 else kT_sb
                            bcol = qkb_sb[:, idx : idx + 1]
                            bswc = qkbs_sb[:, idx : idx + 1]
                            # t = (ps+b)*cos; usw = cross-half sin product,
                            # half-swapped so the combine is one SB+SB add
                            # (PSUM operand base may differ; sign in tgsn)
                            t = t_pool.tile([P, QCH], BF16, name="t")
                            nc.vector.scalar_tensor_tensor(
                                t[:], ps[:], bcol, tgc[:], op0=AX.add, op1=AX.mult
                            )
                            usw = u_pool.tile([P, QCH], BF16, name="usw")
                            for h in range(HPC):
                                r0 = slice(64 * h, 64 * h + 32)
                                r1 = slice(64 * h + 32, 64 * h + 64)
                                nc.vector.scalar_tensor_tensor(
                                    usw[r0, :], ps[r1, :], bswc[r0, :],
                                    tgsn[r0, :], op0=AX.add, op1=AX.mult,
                                )
                                nc.vector.scalar_tensor_tensor(
                                    usw[r1, :], ps[r0, :], bswc[r1, :],
                                    tgsn[r1, :], op0=AX.add, op1=AX.mult,
                                )
                            osl = ssl if dst is kT_sb else slice(0, QCH)
                            nc.vector.tensor_add(dst[:, osl], t[:], usw[:])
                            if debug and sc == 0 and idx == 1:
                                nc.sync.dma_start(
                                    out=dbg["qt"].ap(), in_=state["qTc"][:]
                                )
                                nc.sync.dma_start(
                                    out=dbg["kt"].ap(), in_=kT_sb[:, 0:QCH]
                                )
                        return run

                    def u_v(sb):
                        def run():
                            xr = state["xr"]
                            jb = sc * 4 + sb
                            psv = pps_pool.tile([P, CPC], F32, name="pps")
                            for kc in range(KC):
                                nc.tensor.matmul(
                                    psv[:],
                                    lhsT=xr[:, kc, sb * P : (sb + 1) * P],
                                    rhs=w_sb[:, kc, 2 * CPC : 3 * CPC],
                                    start=(kc == 0),
                                    stop=(kc == KC - 1),
                                )
                            nc.vector.tensor_add(
                                v_sb[:, jb, :]
                                .rearrange("p (h c) -> p h c", h=HPC)[:, :, 0:HD],
                                psv.rearrange("p (h c) -> p h c", h=HPC),
                                vbb_sb.rearrange("p (h c) -> p h c", h=HPC),
                            )
                            if debug and sc == 0 and sb == 0:
                                nc.sync.dma_start(
                                    out=dbg["v"].ap(), in_=v_sb[:, 0, :]
                                )
                        return run

                    units = [u_dma]
                    for idx in range(2):
                        units.append(u_qk_mm(idx, 0, 4))
                        units.append(u_qk_mm(idx, 4, KC))
                        units.append(u_rope(idx))
                    for sb in range(QCH // P):
                        units.append(u_v(sb))
                    return units

                def attn(qc, units=()):
                    from collections import deque

                    units = deque(units)
                    qsl = slice(qc * QCH, (qc + 1) * QCH)
                    qTc = qTs.pop(qc)
                    n_j = n_j_of(qc)
                    ppvs = [
                        ppv_pool.tile([VW, QCH], F32, name="ppv", tag="ppv")
                        for _ in range(HPC)
                    ]
                    per_j = max(1, -(-len(units) // max(1, n_j - 1)))

                    def emit_scores(j):
                        # scores for iteration j, emitted one iteration early
                        # so the PE FIFO never serializes them behind the
                        # exp-blocked PV of the previous iteration
                        ps = sc_pool.tile([P, HPC, QCH], F32, name="ps")
                        for h in range(HPC):
                            hr = slice(64 * h, 64 * h + 64)
                            nc.tensor.matmul(
                                ps[:, h, :],
                                lhsT=kT_sb[hr, j * KCH : (j + 1) * KCH],
                                rhs=qTc[hr, :],
                                start=True,
                                stop=True,
                            )
                        lo = max(0, KCH * j - QCH * qc) if causal else 0
                        if causal and KCH * j >= QCH * qc:
                            nc.vector.tensor_add(
                                ps[:, :, lo : lo + KCH],
                                ps[:, :, lo : lo + KCH],
                                tri_sb[:],
                            )
                        if mask_mode == "general":
                            mt = mload_pool.tile([KCH, QCH], F32, name="mt")
                            nc.sync.dma_start(
                                out=mt[:],
                                in_=maskT.ap()[j * KCH : (j + 1) * KCH, qsl],
                            )
                            for h in range(HPC):
                                nc.vector.tensor_add(
                                    ps[:, h, :], ps[:, h, :], mt[:]
                                )
                        return ps, lo

                    cur = emit_scores(0)
                    for j in range(n_j):
                        ps, lo = cur
                        nxt = emit_scores(j + 1) if j + 1 < n_j else None
                        for _ in range(per_j):
                            if units:
                                units.popleft()()
                        pt = pt_pool.tile([P, HPC, QCH], BF16, name="pt")
                        nc.scalar.activation(
                            pt[:, :, lo:QCH], ps[:, :, lo:QCH],
                            mybir.ActivationFunctionType.Exp, scale=0.125,
                        )
                        if debug and qc == 0 and j == 0:
                            nc.sync.dma_start(out=dbg["pt"].ap(), in_=pt[:])
                        for h in range(HPC):
                            nc.tensor.matmul(
                                ppvs[h][:, lo:QCH],
                                lhsT=v_sb[:, j, h * VW : (h + 1) * VW],
                                rhs=pt[:, h, lo:QCH],
                                start=(j == 0),
                                stop=(j == n_j - 1),
                                skip_group_check=True,
                            )
                        cur = nxt
                    while units:
                        units.popleft()()
                    # Evacuate PSUM promptly: collect denominators on
                    # partition 0 (-64 shift) and copy the unnormalized
                    # attention outputs to SBUF so ppv banks free right away.
                    # The normalize itself (exp(-ln Z) on ScalarE + broadcast
                    # + mul) is deferred into the next chunk's j-loop so it
                    # never blocks the ACT FIFO at the chunk boundary.
                    den2 = rec_pool.tile([1, HPC, QCH], F32, name="den2")
                    atu = atu_pool.tile([P, QCH], BF16, name="atu")
                    for h in range(HPC):
                        nc.vector.tensor_copy(
                            den2[0:1, h, :], ppvs[h][HD : HD + 1, :]
                        )
                        nc.vector.tensor_copy(
                            atu[64 * h : 64 * h + 64, :], ppvs[h][0:HD, :]
                        )
                        if debug and qc == 0 and h == 0:
                            ptmp = ysb_pool.tile([P, QCH], F32, name="ysb")
                            nc.vector.tensor_copy(ptmp[0:VW, :], ppvs[h][:])
                            nc.sync.dma_start(out=dbg["ppv"].ap(), in_=ptmp[:])
                    norms[qc] = (den2, atu)

                def norm_units(qc):
                    den2, atu = norms.pop(qc)
                    units = []

                    def u_rec():
                        lnz = lnz_pool.tile([1, HPC, QCH], F32, name="lnz")
                        nc.scalar.activation(
                            lnz[:], den2[:], mybir.ActivationFunctionType.Ln
                        )
                        rec2 = rec2_pool.tile([1, HPC, QCH], F32, name="rec2")
                        nc.scalar.activation(
                            rec2[:], lnz[:], mybir.ActivationFunctionType.Exp,
                            scale=-1.0,
                        )
                        state = {"rec2": rec2}
                        if debug and qc == 0:
                            nc.sync.dma_start(
                                out=dbg["rec"].ap(), in_=rec2[0:1, 0, :]
                            )
                        return state

                    st = {}

                    def u_first():
                        st.update(u_rec())
                        st["at"] = at_pool.tile([P, QCH], BF16, name="at")
                        ats[qc] = st["at"]

                    def u_mul(h):
                        def run():
                            hr = slice(64 * h, 64 * h + 64)
                            bc = bc_pool.tile([P, QCH], F32, name="bc")
                            nc.gpsimd.partition_broadcast(
                                bc[:], st["rec2"][0:1, h, :]
                            )
                            nc.vector.tensor_mul(
                                st["at"][hr, :], atu[hr, :], bc[hr, :]
                            )
                            if debug and qc == 0 and h == HPC - 1:
                                nc.sync.dma_start(
                                    out=dbg["at"].ap(), in_=st["at"][:]
                                )
                        return run

                    units.append(u_first)
                    for h in range(HPC):
                        units.append(u_mul(h))
                    return units

                def wo_units(qc):
                    # partial output projection, emitted a chunk late and
                    # interleaved so the PE never stalls on the normalize chain
                    at = ats.pop(qc)
                    units = []
                    for sb in range(QCH // P):
                        row0 = qc * QCH + sb * P

                        def run(sb=sb, row0=row0):
                            ysb = ysb_pool.tile([P, D], F32, name="ysb")
                            for nn in range(D // 512):
                                psy = pps_pool.tile([P, 512], F32, name="pps")
                                nc.tensor.matmul(
                                    psy[:],
                                    lhsT=at[:, sb * P : (sb + 1) * P],
                                    rhs=woT_sb[:, nn * 512 : (nn + 1) * 512],
                                    start=True,
                                    stop=True,
                                )
                                nc.vector.tensor_copy(
                                    ysb[:, nn * 512 : (nn + 1) * 512], psy[:]
                                )
                            nc.sync.dma_start(
                                out=y_out.ap()[row0 : row0 + P, :], in_=ysb[:]
                            )

                        units.append(run)
                    return units

                p0 = proj_units(0)
                for u in p0[:7]:  # dma + q/k proj + rope
                    if u is p0[1]:
                        emit_late_consts()
                    u()
                for qc in range(NQC):
                    units = []
                    if qc == 0:
                        units += p0[7:]  # chunk 0 v-projections
                    if qc >= 1:
                        units += norm_units(qc - 1)
                    if qc + 1 < NQC:
                        units += proj_units(qc + 1)
                    if qc >= 2:
                        units += wo_units(qc - 2)
                    attn(qc, units)
                for u in norm_units(NQC - 1):
                    u()
                for u in wo_units(NQC - 2):
                    u()
                for u in wo_units(NQC - 1):
                    u()

    nc.compile()
    _BUILD_CACHE[key] = nc
    return nc


def _detect_mask_mode(mask: np.ndarray):
    m = np.asarray(mask, np.float32).reshape(S, S)
    if not m.any():
        return "none", 0.0, m
    mval = float(m[0, 1])
    if mval < -1e8 and np.array_equal(
        m, np.triu(np.full((S, S), mval, np.float32), 1)
    ):
        return "causal", mval, m
    return "general", 0.0, m


def kernel(
    x, start_pos, freqs_cos, freqs_sin, mask,
    wq_w, wq_b, wk_w, wk_b, wv_w, wv_b, wo_w, wo_b,
):
    x = np.asarray(x, np.float32).reshape(S, D)
    freqs_cos = np.asarray(freqs_cos, np.float32)
    freqs_sin = np.asarray(freqs_sin, np.float32)
    mask_mode, mval, m2d = _detect_mask_mode(np.asarray(mask))

    # pair-split permutation within each head: [0,2,..,62, 1,3,..,63]
    perm1 = np.concatenate([np.arange(0, HD, 2), np.arange(1, HD, 2)])
    perm = np.concatenate([perm1 + h * HD for h in range(HPC)])

    # x^T pre-chunked: [p, sc, kc, t] = x[sc*512+t, kc*128+p]
    xtr = np.ascontiguousarray(
        x.reshape(NQC, QCH, KC, P).transpose(3, 0, 2, 1)
    ).astype(ml_dtypes.bfloat16)

    # trig rows: each 32-row group is cos^T (trigC); trigSN carries the
    # combine sign: [-sin, +sin] per 64-row head block
    cosT = np.ascontiguousarray(freqs_cos.T)  # [32, S]
    sinT = np.ascontiguousarray(freqs_sin.T)
    trigC = np.concatenate([cosT] * 4, axis=0).astype(np.float32)
    trigSN = np.concatenate([-sinT, sinT] * 2, axis=0).astype(np.float32)

    woT_full = np.ascontiguousarray(np.asarray(wo_w, np.float32).T)

    tri2_np = None
    if mask_mode == "causal":
        kk = np.arange(KCH)[:, None]
        qq = np.arange(KCH)[None, :]
        tri1 = np.where(kk > qq, np.float32(8.0 * mval), np.float32(0.0)).astype(
            np.float32
        )
        tri2_np = np.ascontiguousarray(
            np.broadcast_to(tri1[:, None, :], (KCH, 2, KCH))
        )
    maskT_np = None
    if mask_mode == "general":
        maskT_np = np.ascontiguousarray((8.0 * m2d).T.astype(np.float32))

    in_maps = []
    for c in range(NC):
        rows = slice(c * CPC, (c + 1) * CPC)
        wq_s = np.asarray(wq_w, np.float32)[rows, :][perm, :]
        wk_s = np.asarray(wk_w, np.float32)[rows, :][perm, :]
        wv_s = np.asarray(wv_w, np.float32)[rows, :]
        wqkvT = np.concatenate([wq_s.T, wk_s.T, wv_s.T], axis=1).astype(
            ml_dtypes.bfloat16
        )
        # pack [D, 384] -> [p, kc, c] (contiguous per partition)
        wqkvT = wqkvT.reshape(KC, P, 3 * CPC).transpose(1, 0, 2)
        qb = np.asarray(wq_b, np.float32)[rows][perm]
        kb = np.asarray(wk_b, np.float32)[rows][perm]
        vb = np.asarray(wv_b, np.float32)[rows]
        qkb_np = np.stack([qb, kb], axis=1).astype(np.float32)
        # swap the 32-row halves within each 64-row head block
        qkbs_np = np.ascontiguousarray(
            qkb_np.reshape(HPC, 2, 32, 2)[:, ::-1].reshape(P, 2)
        )
        im = {
            "xtr": xtr,
            "wqkvT": np.ascontiguousarray(wqkvT),
            "woT": np.ascontiguousarray(woT_full[rows, :]).astype(ml_dtypes.bfloat16),
            "trigC": trigC,
            "trigSN": trigSN,
            "qkb": qkb_np,
            "qkbs": qkbs_np,
            "vbb": np.broadcast_to(vb, (P, CPC)).copy(),
        }
        if mask_mode == "causal":
            im["tri2"] = tri2_np
        elif mask_mode == "general":
            im["maskT"] = maskT_np
        in_maps.append(im)

    nc = _build(mask_mode)
    res = run_bass_kernel_spmd(nc, in_maps, list(range(NC)))
    y = np.zeros((S, D), np.float64)
    for c in range(NC):
        y += res.results[c]["y"].astype(np.float64)
    y += np.asarray(wo_b, np.float64)
    return y.reshape(B, S, D).astype(np.float32)
